# revision 28
# baseline (speedup 1.0000x reference)
"""Trainium2 Bass kernel for nn_BagKQMClassModel.

Computation (per batch item b):
    K[b,n,m]   = exp(-d2/(2 s^2)),  d2 = |A[b,n] - C[m]|^2
    out_w[b,m] = (1/N) sum_n comp_w[m] * K^2
    y_w        = out_w / sum_m out_w
    probs      = y_w @ (y_v^2),  y_v = c_y rows normalized

Key transformations:
  * K^2 = exp(-d2/s^2): one exp per (b,n,m) element.
  * d2 folded into one fp16 matmul with augmented contraction (34 rows):
        rows 0-31: data a_d * c_d;  row 32: CT 1, AT -a2/2;
        row 33: CT -b2/2, AT 1
    so exp arg = (2/s^2) * g with no ACT bias.
  * probs = T[:, :10] / T[:, 10], T = sum_{n,m} K2 * W with
    W[m, :10] = comp_w[m] * c_y[m]^2 / |c_y[m]|^2, W[m, 10] = comp_w[m].
  * m and (b,n) are PERMUTED vs the reference layout so every input DMA
    is contiguous (partition p holds a contiguous DRAM block):
    m = 16p + r;  bn: p = 4t + n//32, j = n%32.  All downstream sums are
    permutation-invariant since c_y/comp_w use the same m-permutation.
  * One PSUM tile S[11, 128] accumulates mm2 over all 64 (m-chunk,
    j-block) steps; the final per-batch output is one [11, 32, 4]
    free-dim reduce + an 11x32 transpose.
  * mm2 runs as fp8e4 DoubleRow over CHUNK PAIRS (stationary W8
    [128, 2, 11], moving r3 [128, 2, 128]) - half the mm2 instruction
    count and half the stream cycles.  W8 is pre-scaled by 2048 so the
    tiny comp_w values stay inside fp8e4's dynamic range (min subnormal
    2^-9); the scale cancels in the final T[:, :10] / T[:, 10].
  * exp runs on 1024-wide ACT windows from a 2-deep PSUM ring (2 banks
    each; + 1 bank S + 3 x 1 bank transpose scratch = 8 banks).
  * The 8:1 bag reduction batches SIX chunks into one DVE op per
    halving level, amortizing the ~151-cycle DVE op overhead.
  * CT chunks 2-15, AT j-blocks 1-3, and the W build all run INSIDE the
    main loop in PE/DVE slack (deadline-scheduled quanta).

Sharding: batch 256 -> 32 items per core across 8 cores; c_x/c_y/comp_w
replicated. No collectives (forward only).
"""

import numpy as np

import concourse.bacc as bacc
import concourse.mybir as mybir
import concourse.tile as tile
from concourse.bass_utils import run_bass_kernel_spmd
from concourse.masks import make_identity

NCORES = 8
BS, N, DX, DY, M = 256, 128, 32, 10, 2048
BPC = BS // NCORES      # 32 batch items per core
MB = M // 128           # 16 chunks of the component axis
KAUG = DX + 2           # 34 augmented contraction rows
NJ = 32                 # bn = 32 j-slices x 128 p
JB = 8                  # j slices per block
NBLK = NJ // JB         # 4 blocks
F_CHUNK = JB * 128      # 1024 bn columns per (m-chunk, j-block) step
WIN = 1024              # ACT window = one chunk (2 x 512-col mm1 pieces)
NRING = 12              # K2 ring slots (6-chunk reduce groups never wrap)
GRP = 6                 # chunks per DVE reduce group
WSCL = 2048.0           # fp8 mm2 weight prescale (cancels in the division)
MIN_SIGMA = 1e-3
FP32 = mybir.dt.float32
FP16 = mybir.dt.float16
FP8 = mybir.dt.float8e4
AX = mybir.AxisListType
ALU = mybir.AluOpType
ACTF = mybir.ActivationFunctionType
DR = mybir.MatmulPerfMode.DoubleRow


def _body(tc, inp, cx, cy, cw_d, out_d, scale):
    nc = tc.nc
    from contextlib import ExitStack

    with ExitStack() as ctx:
        const = ctx.enter_context(tc.tile_pool(name="const", bufs=1))
        work = ctx.enter_context(tc.tile_pool(name="work", bufs=2))
        psum = ctx.enter_context(tc.tile_pool(name="psum", bufs=1, space="PSUM"))

        # ---- contiguous input DMAs ----------------------------------------
        # c_x first on the sync queue: it gates the whole prologue chain.
        # A arrives in quarters so block 0's j-slices land earliest.
        cx_nat = const.tile([128, MB, DX], FP32)    # m = 16p + r
        nc.sync.dma_start(out=cx_nat, in_=cx.rearrange("(p r) d -> p r d", p=128))
        A_nat = const.tile([128, NJ, DX], FP32)     # p = 4t + n//32, j = n%32
        inp_r = inp.rearrange("t (a j) d -> (t a) j d", a=4)
        nc.sync.dma_start(out=A_nat[:, 0:8, :], in_=inp_r[:, 0:8, :])
        nc.scalar.dma_start(out=A_nat[:, 8:16, :], in_=inp_r[:, 8:16, :])
        nc.sync.dma_start(out=A_nat[:, 16:24, :], in_=inp_r[:, 16:24, :])
        nc.scalar.dma_start(out=A_nat[:, 24:32, :], in_=inp_r[:, 24:32, :])
        cy_nat = const.tile([128, MB, DY], FP32)
        nc.gpsimd.dma_start(out=cy_nat, in_=cy.rearrange("(p r) d -> p r d", p=128))
        cw_nat = const.tile([128, MB], FP32)
        nc.gpsimd.dma_start(out=cw_nat, in_=cw_d.rearrange("(p r) -> p r", p=128))

        ident16 = const.tile([128, 128], FP16)
        make_identity(nc, ident16)
        ident32 = const.tile([DY + 1, DY + 1], FP32)
        make_identity(nc, ident32)

        # preload the exp table set during the prologue DMA wait
        warm = const.tile([128, 1], FP32)
        nc.gpsimd.memset(warm, 0.0)
        warm2 = const.tile([128, 1], FP32)
        nc.scalar.activation(warm2, warm, ACTF.Exp, bias=0.0, scale=1.0)

        # packed fp16 transpose sources
        cx_pack = const.tile([128, MB, KAUG], FP16)  # [d x32, 1.0, -b2/2]
        A_pack = const.tile([128, NJ, KAUG], FP16)   # [d x32, -a2/2, 1.0]
        nc.gpsimd.memset(cx_pack[:, :, DX : DX + 1], 1.0)
        nc.gpsimd.memset(A_pack[:, :, DX + 1 : DX + 2], 1.0)

        CT16 = const.tile([KAUG, MB, 128], FP16)
        AT16 = const.tile([KAUG, NJ, 128], FP16)
        W8 = const.tile([128, MB // 2, 2, 32], FP8)   # chunk-pair, padded to 32 cols
        nc.gpsimd.memset(W8, 0.0)
        S = psum.tile([32, 128], FP32, tag="S")
        K2r = const.tile([128, NRING, WIN], FP16)
        K2f = K2r.rearrange("p w f -> p (w f)")

        one3 = lambda t: t.rearrange("p (s o) -> p s o", o=1)

        # ---- prep helpers --------------------------------------------------
        def quant_chain(nat, pack, lo, hi, colh):
            sq = work.tile([128, NJ, DX], FP32, tag="sq", bufs=2)
            sqv = sq[:, 0 : hi - lo, :]
            nc.vector.tensor_mul(sqv, nat[:, lo:hi, :], nat[:, lo:hi, :])
            mh = work.tile([128, NJ], FP32, tag="mh", bufs=2)
            mhv = mh[:, 0 : hi - lo]
            nc.vector.tensor_reduce(out=one3(mhv), in_=sqv, axis=AX.X, op=ALU.add)
            nc.vector.tensor_scalar_mul(mhv, mhv, -0.5)
            nc.vector.tensor_copy(pack[:, lo:hi, colh : colh + 1], one3(mhv))
            nc.vector.tensor_copy(pack[:, lo:hi, 0:DX], nat[:, lo:hi, :])

        def transpose_one(pack, dst, idx, use_scalar):
            trk = psum.tile([KAUG, 128], FP16, tag="trk", bufs=3)
            nc.tensor.transpose(trk, pack[:, idx, :], ident16)
            if use_scalar:
                nc.scalar.copy(dst[:, idx, :], trk)
            else:
                nc.vector.tensor_copy(dst[:, idx, :], trk)

        def w_chain():
            sqy = work.tile([128, MB, DY], FP32, tag="sqy")
            nc.vector.tensor_mul(sqy, cy_nat, cy_nat)
            ssum = work.tile([128, MB], FP32, tag="ssum")
            nc.vector.tensor_reduce(out=one3(ssum), in_=sqy, axis=AX.X, op=ALU.add)
            rec = work.tile([128, MB], FP32, tag="rec")
            nc.vector.reciprocal(rec, ssum)
            facr = work.tile([128, MB], FP32, tag="facr")
            nc.vector.tensor_mul(facr, rec, cw_nat)
            facr_b = one3(facr).broadcast_to([128, MB, DY])
            wtmp = work.tile([128, MB, DY], FP32, tag="wtmp")
            nc.vector.tensor_mul(wtmp, sqy, facr_b)
            w8f = W8.rearrange("p a h c -> p (a h) c")     # [128, 16, 32]
            nc.vector.tensor_scalar_mul(w8f[:, :, 0:DY], wtmp, WSCL)
            nc.vector.tensor_scalar_mul(w8f[:, :, DY : DY + 1], one3(cw_nat), WSCL)

        # ---- prologue: cx chunks 0-1, AT block 0, then the rest of cx ----
        # (the bulk cx chain is only needed by loop step 2's CT transpose, so
        # it runs AFTER block 0's chain to pull the first mm1 earlier)
        quant_chain(cx_nat, cx_pack, 0, 2, DX + 1)
        for c in range(2):
            transpose_one(cx_pack, CT16, c, True)
        quant_chain(A_nat, A_pack, 0, JB // 2, DX)
        for j in range(JB // 2):
            transpose_one(A_pack, AT16, j, True)
        quant_chain(A_nat, A_pack, JB // 2, JB, DX)
        for j in range(JB // 2, JB):
            transpose_one(A_pack, AT16, j, True)
        quant_chain(cx_nat, cx_pack, 2, MB, DX + 1)

        # deferred prep, deadline-ordered: CT chunk c used at step c; AT
        # block b used from step 16b; W used by the first mm2 drain (~s8).
        def ct_q(c):
            return lambda: transpose_one(cx_pack, CT16, c, False)

        def at_q(j):
            return lambda: transpose_one(A_pack, AT16, j, False)

        def chain_q(jlo):
            return lambda: quant_chain(A_nat, A_pack, jlo, jlo + JB, DX)

        quanta = [ct_q(2), ct_q(3), ct_q(4), w_chain, ct_q(5), chain_q(JB)]
        for c in range(6, 14):
            quanta += [ct_q(c), at_q(c + 2)]     # AT block 1: j = 8..15
        quanta += [ct_q(14), ct_q(15)]
        for blk in (2, 3):
            jlo = blk * JB
            quanta.append(chain_q(jlo))
            quanta += [at_q(j) for j in range(jlo, jlo + JB)]
        # 2 quanta/step for the first 13 steps, then 1/step (done by s28).
        sched = {s: [] for s in range(64)}
        qi = 0
        for s in range(64):
            take = min(2 if s < 13 else 1, len(quanta) - qi)
            sched[s] = quanta[qi : qi + take]
            qi += take
        assert qi == len(quanta), (qi, len(quanta))

        AT16f = AT16.rearrange("p j c -> p (j c)")
        TOT = NBLK * MB

        # ---- main loop ----------------------------------------------------
        def emit_reduce(grp_base, nch):
            kv = K2f[:, grp_base : grp_base + nch * F_CHUNK].rearrange(
                "p (t f) -> p t f", t=nch
            )
            r1 = work.tile([128, GRP, 512], FP16, tag="r1")
            r1v = r1[:, 0:nch, :]
            nc.vector.tensor_add(r1v, kv[:, :, 0:512], kv[:, :, 512:1024])
            r2 = work.tile([128, GRP, 256], FP16, tag="r2")
            r2v = r2[:, 0:nch, :]
            nc.vector.tensor_add(r2v, r1v[:, :, 0:256], r1v[:, :, 256:512])
            r3 = work.tile([128, GRP, 128], FP16, tag="r3", bufs=2)
            r3v = r3[:, 0:nch, :]
            nc.vector.tensor_add(r3v, r2v[:, :, 0:128], r2v[:, :, 128:256])
            r38 = work.tile([128, GRP, 128], FP8, tag="r38", bufs=3)
            nc.vector.tensor_copy(r38[:, 0:nch, :], r3v)
            return r38

        def emit_mm2(entry):
            r38t, s0, nch = entry
            for i in range(0, nch, 2):
                s_ = s0 + i
                pr = (s_ % MB) // 2
                nc.tensor.matmul(
                    S,
                    W8[:, pr, :, :],
                    r38t[:, i : i + 2, :],
                    start=(s_ == 0),
                    stop=(s_ + 1 == TOT - 1),
                    perf_mode=DR,
                )
            return s0 + nch

        # p-state pre-warm: ~5us of back-to-back matmuls right before the
        # loop so the PE enters the loop at its ramped clock (measured:
        # ramp engages after ~10 gapless matmuls).
        warm_mov = ident16.rearrange("p (o c) -> p o c", o=1).broadcast_to(
            [128, 4, 128]
        )
        for _ in range(18):
            gwu = psum.tile([128, WIN], FP32, tag="g", bufs=2, name="gwu")
            nc.tensor.matmul(gwu[:, 0:512], ident16, warm_mov, start=True, stop=True)

        pending = []
        mm2_done = 0
        for s in range(TOT):
            jb, c = divmod(s, MB)
            gw = psum.tile([128, WIN], FP32, tag="g", bufs=2, name="gw")
            for q in range(2):
                nc.tensor.matmul(
                    gw[:, q * 512 : (q + 1) * 512],
                    CT16[:, c, :],
                    AT16f[:, jb * F_CHUNK + q * 512 : jb * F_CHUNK + (q + 1) * 512],
                    start=True,
                    stop=True,
                )
            nc.scalar.activation(
                K2r[:, s % NRING, :], gw, ACTF.Exp, bias=0.0, scale=scale
            )
            if s % GRP == GRP - 1 and s < GRP * (TOT // GRP):
                grp = s // GRP
                r38t = emit_reduce(WIN * ((GRP * grp) % NRING), GRP)
                pending.append((r38t, GRP * grp, GRP))
            if s == 61:
                # first tail pair reduces while chunks 62-63 still exp
                pending.append((emit_reduce(WIN * (60 % NRING), 2), 60, 2))
            while pending and pending[0][1] + pending[0][2] + 2 <= s:
                mm2_done = emit_mm2(pending.pop(0))
            for fn in sched[s]:
                fn()
        # tail: final 2-chunk pair (ring slots 2..3)
        pending.append((emit_reduce(WIN * (62 % NRING), 2), 62, 2))
        for entry in pending:
            mm2_done = emit_mm2(entry)
        assert mm2_done == TOT

        # ---- epilogue: T = reduce(S), probs = T[:, :10] / T[:, 10] --------
        Tred = const.tile([DY + 1, BPC], FP32)
        nc.vector.tensor_reduce(
            out=Tred.rearrange("p (t o) -> p t o", o=1),
            in_=S[0 : DY + 1, :].rearrange("p (t f) -> p t f", f=4),
            axis=AX.X,
            op=ALU.add,
        )
        trT = psum.tile([BPC, DY + 1], FP32, tag="trk", bufs=3)
        nc.tensor.transpose(trT, Tred, ident32)
        Tt = const.tile([BPC, DY + 1], FP32)
        nc.vector.tensor_copy(Tt, trT)
        recd = const.tile([BPC, 1], FP32)
        nc.vector.reciprocal(recd, Tt[:, DY : DY + 1])
        outsb = const.tile([BPC, DY], FP32)
        nc.vector.tensor_scalar(
            out=outsb, in0=Tt[:, 0:DY], scalar1=recd, scalar2=None, op0=ALU.mult
        )
        nc.sync.dma_start(out=out_d, in_=outsb)


def build_program(scale):
    nc = bacc.Bacc(
        "TRN2",
        target_bir_lowering=False,
        debug=False,
        enable_asserts=False,
        num_devices=NCORES,
    )
    inp = nc.dram_tensor("inputs", [BPC, N, DX], FP32, kind="ExternalInput").ap()
    cx = nc.dram_tensor("c_x", [M, DX], FP32, kind="ExternalInput").ap()
    cy = nc.dram_tensor("c_y", [M, DY], FP32, kind="ExternalInput").ap()
    cw = nc.dram_tensor("comp_w", [M], FP32, kind="ExternalInput").ap()
    out = nc.dram_tensor("out", [BPC, DY], FP32, kind="ExternalOutput").ap()
    with tile.TileContext(nc) as tc:
        _body(tc, inp, cx, cy, cw, out, scale)
    nc.compile()
    return nc


_PROGRAM_CACHE: dict = {}


def _get_program(scale):
    nc = _PROGRAM_CACHE.get(scale)
    if nc is None:
        nc = build_program(scale)
        _PROGRAM_CACHE[scale] = nc
    return nc


def make_in_maps(inputs, c_x, c_y, comp_w):
    shards = np.ascontiguousarray(inputs.reshape(NCORES, BPC, N, DX))
    return [
        {
            "inputs": shards[i],
            "c_x": np.ascontiguousarray(c_x),
            "c_y": np.ascontiguousarray(c_y),
            "comp_w": np.ascontiguousarray(comp_w),
        }
        for i in range(NCORES)
    ]


def scale_from_sigma(sigma) -> float:
    s = max(float(np.asarray(sigma, dtype=np.float64)), MIN_SIGMA)
    return float(2.0 / (s * s))


def kernel(inputs, sigma, c_x, c_y, comp_w, _run_kwargs=None):
    nc = _get_program(scale_from_sigma(sigma))
    in_maps = make_in_maps(inputs, c_x, c_y, comp_w)
    res = run_bass_kernel_spmd(
        nc, in_maps, core_ids=list(range(NCORES)), **(_run_kwargs or {})
    )
    out = np.concatenate([res.results[i]["out"] for i in range(NCORES)], axis=0)
    return out.astype(np.float32)
